# revision 9
# baseline (speedup 1.0000x reference)
"""SSIM loss kernel v7 for Trainium2 (8 NeuronCores, batch-parallel).

Math: reference SSIM, 11x11 box filters, VALID padding, (16,3,512,512)
fp32 pairs -> scalar 1 - mean(ssim_map).  2 batch items = 6 planes/core.
Inputs are rounded to bf16 on the host (shifts the loss by ~1e-6 rel,
measured) so the DMA loads move half the bytes.

Structure per plane (pad-free [128, 4*512] segment layout; H-window sums
never cross image-row boundaries because the 10 boundary-window outputs
fall outside the matmul rhs slice [512k+10 : 512k+512]):
  - xx = x^2, yy = y^2 (Act Square), u = xx+yy (DVE TT bf16),
    v = x*y (Pool TT)
  - horizontal 11-window sums of x, y, u, v via DVE tensor_tensor_scan
  - vertical 11-window sums via PE band matmuls with PSUM butterflies:
      Sig  = wv*(Hx + Hy)   = wv121*(mu_x+mu_y)
      Del  = wv*(Hy - Hx)   = wv121*(mu_y-mu_x)
      SQ   = wv*Hu          = wv121*(E[x^2]+E[y^2])
      DQ   = wv*Hv          = wv121*E[xy]
  - chain: sa=Sig^2, sb=Del^2 (Act); A1=sa-sb, A2=sa+sb (Pool TT);
    num = (A1+c1)*(DQ - C1n*A1 + C2n)   [one custom DVE op, PSUM in1]
    den = (A2+c1)*(SQ - C1d*A2 + C2d)   [same op, different consts]
    r = Reciprocal(den) on Act; ssim_raw = num*r (Pool TT) = ssim/2;
    accumulate via Act Copy+accum into per-(plane,h) columns.

Scale bookkeeping (wv = 2^-7 exact in bf16, wv121 = 121/128):
  A1 = 4*wv121^2*mu_x*mu_y ; A2 = 2*wv121^2*(mu_x^2+mu_y^2) ; K = 2*wv121^2
  num = K*(2 mu_x mu_y + C1) * (wv121/2)*(2 sigma_xy + C2)
  den = K*(mu_x^2+mu_y^2+C1) * wv121*(sigma_x^2+sigma_y^2+C2)
  num/den = ssim/2  -> host multiplies the accumulated sum by 2.
"""

import sys
from contextlib import ExitStack

import numpy as np

sys.path.insert(0, "/opt/trn_rl_repo")

import ml_dtypes  # noqa: E402

import concourse.bass as bass  # noqa: E402
import concourse.tile as tile  # noqa: E402
from concourse import bacc, bass_utils, mybir  # noqa: E402
import concourse.dve_ops as dve_ops  # noqa: E402
from concourse.dve_spec import Spec, Src0, Src1, C0, C1, C2, lower  # noqa: E402
from concourse.dve_uop import DveOpSpec  # noqa: E402

F32 = mybir.dt.float32
BF16 = mybir.dt.bfloat16
ALU = mybir.AluOpType
ACTF = mybir.ActivationFunctionType

WIN = 11
IMG = 512
OUT = IMG - WIN + 1  # 502
NSEG = 4
BUF = WIN + NSEG * IMG  # 2059: one leading 11-col zero pad
NPLANE = 6
NCORES = 8

C1_SSIM = (0.01 * 1.0) ** 2
C2_SSIM = (0.03 * 1.0) ** 2
WV = float(2.0 ** -7)  # band weight, bf16-exact
WV121 = 121.0 / 128.0
KS = 2.0 * WV121 * WV121
CONST_C1 = float(KS * C1_SSIM)            # s0 for both halves
C1N = float(1.0 / (4.0 * WV121))          # s1 of num half
C2N = float((WV121 / 2.0) * C2_SSIM)      # imm2 of num half
C1D = float(1.0 / (2.0 * WV121))          # s1 of den half
C2D = float(WV121 * C2_SSIM)              # imm2 of den half

# (m, k) -> index into the weight block array
_PAIRS = [(0, 0), (0, 1), (1, 1), (1, 2), (2, 2), (2, 3), (3, 3)]
_WIDX = {mk: i for i, mk in enumerate(_PAIRS)}


def _mk_op(name, spec):
    for o in dve_ops.OPS:
        if o.name == name:
            return o
    opcode = dve_ops._CUSTOM_DVE_ROW_BASE + len(dve_ops.OPS)
    shas = {}
    for ver in ("v3", "v4"):
        try:
            compiled = DveOpSpec(
                name=name, opcode=opcode, uops=lower(spec, ver=ver),
                rd1_en=True)
            shas[ver] = compiled.sha(ver)
        except Exception:
            pass
    op = dve_ops.DveOp(name, spec, subdim=False, uops_sha=shas)
    dve_ops.OPS.append(op)
    dve_ops.CUSTOM_DVE_SPECS[name] = spec
    dve_ops._SUB_OPCODE_FOR_NAME[name] = opcode
    return op


from concourse.dve_spec import sq as _sq  # noqa: E402

# out = in0^2 + in1^2 + s0
OP_SUMSQ = _mk_op(
    "SUMSQ_ANT",
    Spec(
        body=_sq(Src0) + _sq(Src1) + C0,
        reference=lambda in0, in1, s0, s1, imm2: (
            in0.astype(np.float32) ** 2 + in1.astype(np.float32) ** 2 + s0
        ),
    ),
)

# out = in0*in1 + s0
OP_MULADD = _mk_op(
    "MULADD_ANT",
    Spec(
        body=Src0 * Src1 + C0,
        reference=lambda in0, in1, s0, s1, imm2: (
            in0.astype(np.float32) * in1.astype(np.float32) + s0
        ),
    ),
)

# out = (in0 + s0) * (in1 - in0*s1 + imm2)
OP_SSIM_HALF2 = _mk_op(
    "SSIM_HALF2_ANT",
    Spec(
        body=(Src0 + C0) * (Src1 - Src0 * C1 + C2),
        reference=lambda in0, in1, s0, s1, imm2: (
            (in0.astype(np.float32) + s0)
            * (in1.astype(np.float32) - in0.astype(np.float32) * s1 + imm2)
        ),
    ),
)

# out = (in0 + s0) * (in1 - in0*s1)  (C2 folded into the v map)
OP_SSIM_HALF = _mk_op(
    "SSIM_HALF_ANT",
    Spec(
        body=(Src0 + C0) * (Src1 - Src0 * C1),
        reference=lambda in0, in1, s0, s1, imm2: (
            (in0.astype(np.float32) + s0)
            * (in1.astype(np.float32) - in0.astype(np.float32) * s1)
        ),
    ),
)


def _build_weights() -> np.ndarray:
    w = np.zeros((2, len(_PAIRS), 128, 128), dtype=np.float32)
    for idx, (m, k) in enumerate(_PAIRS):
        for i in range(128):
            for o in range(128):
                d = (128 * k + i) - (128 * m + o)
                if 0 <= d < WIN:
                    w[0, idx, i, o] = WV
                    w[1, idx, i, o] = -WV
    return w.astype(ml_dtypes.bfloat16)


def _ktiles(m):
    return [m] if m == 3 else [m, m + 1]


def _act_recip(nc, out, in_):
    """Reciprocal on the Act engine (bass blocks nc.scalar.activation for it;
    emit InstActivation directly: ins = in_, bias, scale, alpha)."""
    eng = nc.scalar
    ins = [eng.lower_ap(in_)]
    for v in (0.0, 1.0, 0.0):
        ins.append(mybir.ImmediateValue(dtype=mybir.dt.float32, value=float(v)))
    return eng.add_instruction(
        mybir.InstActivation(
            name=eng.bass.get_next_instruction_name(),
            func=ACTF.Reciprocal,
            ins=ins,
            outs=[eng.lower_ap(out)],
        )
    )


def _kernel_body(ctx: ExitStack, tc: tile.TileContext, x_d, y_d, wv_d, acc_d):
    nc = tc.nc

    singles = ctx.enter_context(tc.tile_pool(name="singles", bufs=1))
    xy_pool = ctx.enter_context(tc.tile_pool(name="xy", bufs=4))
    uv_pool = ctx.enter_context(tc.tile_pool(name="uv", bufs=2))
    h_pool = ctx.enter_context(tc.tile_pool(name="hmaps", bufs=2))
    ch_pool = ctx.enter_context(tc.tile_pool(name="chain", bufs=3))
    psum_pool = ctx.enter_context(tc.tile_pool(name="ps", bufs=1, space="PSUM"))

    wv_sb = singles.tile([128, 2 * len(_PAIRS), 128], BF16)
    acc_sb = singles.tile([128, 32], F32)
    nc.vector.memset(acc_sb[:], 0.0)

    def emit_load(p):
        # load only the high 16 bits of each fp32 (bf16 truncation):
        # halves DMA bytes; shifts the loss by ~-2.5e-5 rel (measured)
        xb = xy_pool.tile([128, BUF], BF16, tag="xb")
        yb = xy_pool.tile([128, BUF], BF16, tag="yb")
        nc.gpsimd.memset(xb[:, 0:WIN], 0.0)
        nc.gpsimd.memset(yb[:, 0:WIN], 0.0)
        nc.sync.dma_start(
            out=xb[:, WIN:BUF].rearrange("q (s c) -> q s c", s=NSEG),
            in_=x_d[p].rearrange("(s q) w -> q s w", q=128),
        )
        nc.sync.dma_start(
            out=yb[:, WIN:BUF].rearrange("q (s c) -> q s c", s=NSEG),
            in_=y_d[p].rearrange("(s q) w -> q s w", q=128),
        )
        return xb, yb

    def emit_pre(p, xb, yb):
        # u = x^2+y^2 (Act squares + Pool add; C2D rides den's imm2);
        # v = x*y+C2/2 (custom; const rides the linear filter into num)
        xx = uv_pool.tile([128, BUF], BF16, tag="xx")
        yy = uv_pool.tile([128, BUF], BF16, tag="yy")
        ub = uv_pool.tile([128, BUF], BF16, tag="ub")
        vb = uv_pool.tile([128, BUF], BF16, tag="vb")
        nc.gpsimd.tensor_mul(vb[:], xb[:], yb[:])
        nc.scalar.activation(out=xx[:], in_=xb[:], func=ACTF.Square)
        nc.scalar.activation(out=yy[:], in_=yb[:], func=ACTF.Square)
        nc.gpsimd.tensor_add(ub[:], xx[:], yy[:])
        return ub, vb

    def emit_scans(xb, yb, ub, vb):
        hx = h_pool.tile([128, BUF], BF16, tag="hx")
        hy = h_pool.tile([128, BUF], BF16, tag="hy")
        hu = h_pool.tile([128, BUF], BF16, tag="hu")
        hv = h_pool.tile([128, BUF], BF16, tag="hv")
        for src_t, dst in ((xb, hx), (yb, hy), (vb, hv), (ub, hu)):
            nc.vector.tensor_tensor_scan(
                out=dst[:, WIN:BUF],
                data0=src_t[:, WIN:BUF],
                data1=src_t[:, 0:BUF - WIN],
                initial=0.0,
                op0=ALU.add,
                op1=ALU.subtract,
            )
        return hx, hy, hu, hv

    def emit_chain(p, hx, hy, hu, hv):
        for h in (0, 1):
            W2 = 2 * OUT  # 1004
            psS = psum_pool.tile([128, 2, 512], F32, tag="psS")
            psD = psum_pool.tile([128, 2, 512], F32, tag="psD")
            psQ = psum_pool.tile([128, 2, 512], F32, tag="psQ")
            psV = psum_pool.tile([128, 2, 512], F32, tag="psV")

            def band(ps, local, groups):
                m = 2 * h + local
                ks = _ktiles(m)
                n = sum(len(ks) for _ in groups)
                i = 0
                for hmap, cls in groups:
                    for k in ks:
                        nc.tensor.matmul(
                            ps[:, local, 0:OUT],
                            wv_sb[:, cls * len(_PAIRS) + _WIDX[(m, k)], :],
                            hmap[:, IMG * k + 2 * WIN - 1: IMG * k + WIN + IMG],
                            start=(i == 0),
                            stop=(i == n - 1),
                        )
                        i += 1

            for local in (0, 1):
                band(psQ, local, [(hu, 0)])
            for local in (0, 1):
                band(psV, local, [(hv, 0)])
            for local in (0, 1):
                band(psS, local, [(hx, 0), (hy, 0)])
            for local in (0, 1):
                band(psD, local, [(hx, 1), (hy, 0)])

            sa = ch_pool.tile([128, W2], BF16, tag="sa")
            sb_ = ch_pool.tile([128, W2], BF16, tag="sb")
            sa3 = sa[:].rearrange("q (l c) -> q l c", l=2)
            sb3 = sb_[:].rearrange("q (l c) -> q l c", l=2)
            nc.scalar.activation(out=sa3, in_=psS[:, :, 0:OUT], func=ACTF.Square)
            nc.scalar.activation(out=sb3, in_=psD[:, :, 0:OUT], func=ACTF.Square)

            A1 = ch_pool.tile([128, W2], BF16, tag="A1")
            A2 = ch_pool.tile([128, W2], BF16, tag="A2")
            nc.vector.tensor_sub(A1[:], sa[:], sb_[:])
            nc.gpsimd.tensor_add(A2[:], sa[:], sb_[:])

            num = ch_pool.tile([128, W2], BF16, tag="num")
            den = ch_pool.tile([128, W2], BF16, tag="den")
            for local in (0, 1):
                sl = slice(OUT * local, OUT * (local + 1))
                nc.vector._custom_dve(
                    OP_SSIM_HALF2, out=num[:, sl], in0=A1[:, sl],
                    in1=psV[:, local, 0:OUT],
                    s0=CONST_C1, s1=C1N, imm2=C2N,
                )
            for local in (0, 1):
                sl = slice(OUT * local, OUT * (local + 1))
                nc.vector._custom_dve(
                    OP_SSIM_HALF2, out=den[:, sl], in0=A2[:, sl],
                    in1=psQ[:, local, 0:OUT],
                    s0=CONST_C1, s1=C1D, imm2=C2D,
                )

            rcp = ch_pool.tile([128, W2], BF16, tag="rcp")
            _act_recip(nc, rcp[:], den[:])
            ssim = ch_pool.tile([128, W2], BF16, tag="ssim")
            nc.vector.tensor_mul(ssim[:], num[:], rcp[:])

            col = 3 * p + 2 * h
            if h == 0:
                nc.scalar.activation(
                    out=ssim[:], in_=ssim[:], func=ACTF.Copy, scale=1.0,
                    accum_out=acc_sb[:, col:col + 1],
                )
            else:
                nc.scalar.activation(
                    out=ssim[:, 0:OUT], in_=ssim[:, 0:OUT], func=ACTF.Copy,
                    scale=1.0, accum_out=acc_sb[:, col:col + 1],
                )
                mp = 118
                nc.scalar.activation(
                    out=ssim[:mp, OUT:W2], in_=ssim[:mp, OUT:W2],
                    func=ACTF.Copy, scale=1.0,
                    accum_out=acc_sb[:mp, col + 1:col + 2],
                )

    staged = {}
    scanned = {}
    for p in range(NPLANE + 2):
        if p < NPLANE:
            staged[p] = emit_load(p)
        if p == 0:
            # weights load deferred so plane-0 x/y DMAs go first
            nc.gpsimd.dma_start(
                out=wv_sb[:], in_=wv_d.rearrange("c k i o -> i (c k) o"))
        if 1 <= p <= NPLANE:
            xb, yb = staged.pop(p - 1)
            ub, vb = emit_pre(p - 1, xb, yb)
            scanned[p - 1] = emit_scans(xb, yb, ub, vb)
        if p >= 2:
            emit_chain(p - 2, *scanned.pop(p - 2))

    nc.sync.dma_start(out=acc_d, in_=acc_sb[:])


_CACHE = {}


def _get_nc():
    if "nc" in _CACHE:
        return _CACHE["nc"]
    nc = bacc.Bacc("TRN2", target_bir_lowering=False, debug=False)
    x_d = nc.dram_tensor("x", [NPLANE, IMG, IMG], BF16, kind="ExternalInput").ap()
    y_d = nc.dram_tensor("y", [NPLANE, IMG, IMG], BF16, kind="ExternalInput").ap()
    wv_d = nc.dram_tensor(
        "wv", [2, len(_PAIRS), 128, 128], BF16, kind="ExternalInput"
    ).ap()
    acc_d = nc.dram_tensor("acc", [128, 32], F32, kind="ExternalOutput").ap()
    with tile.TileContext(nc) as tc, ExitStack() as ctx:
        _kernel_body(ctx, tc, x_d, y_d, wv_d, acc_d)
    nc.compile()
    _CACHE["nc"] = nc
    return nc


def _run(x, y, trace=False, **kw):
    nc = _get_nc()
    wv = _build_weights()
    x = np.ascontiguousarray(np.asarray(x), dtype=np.float32)
    y = np.ascontiguousarray(np.asarray(y), dtype=np.float32)
    b_per = x.shape[0] // NCORES
    in_maps = []
    for c in range(NCORES):
        xs = x[c * b_per:(c + 1) * b_per].reshape(NPLANE, IMG, IMG)
        ys = y[c * b_per:(c + 1) * b_per].reshape(NPLANE, IMG, IMG)
        xs = xs.astype(ml_dtypes.bfloat16)
        ys = ys.astype(ml_dtypes.bfloat16)
        in_maps.append({"x": xs, "y": ys, "wv": wv})
    res = None
    for attempt in range(3):
        try:
            res = bass_utils.run_bass_kernel_spmd(
                nc, in_maps, core_ids=list(range(NCORES)), trace=trace, **kw
            )
            break
        except Exception:
            if attempt == 2:
                raise
            import time as _time
            _time.sleep(2.0)
    total = 0.0
    for r in res.results:
        total += r["acc"].astype(np.float64).sum()
    # acc holds ssim/2 sums -> multiply by 2
    mean = 2.0 * total / float(16 * 3 * OUT * OUT)
    out = np.float32(1.0 - mean)
    return out, res


def kernel(x, y):
    out, _ = _run(x, y, trace=False)
    return out
